# revision 2
# baseline (speedup 1.0000x reference)
"""IDW k-NN flow interpolation via anchor-pruned candidate lists on 8
Trainium2 NeuronCores.

Sharding: queries split 8 ways (4096/core); anchor/table data replicated.

Algorithm (validated bit-exact in hostsim.py, rel ~1.1e-2):
  Host: k-means C=2048 anchors over refs; per-anchor list of the L=256
  refs nearest its centroid (gather table row, planar pts fp32 + flow
  fp32); per-anchor gamma = 1/RL^2 for scale-free normalized scoring.
  Device, per 128-query tile:
   A. PE bf16 packed matmul scores all anchors: P = OFF -
      round(2^s * |q-c|^2/RL_c^2)*2^11 + anchor_idx (C1-snap trick),
      DVE max-reduce -> winning anchor's packed score; idx = P mod 2048.
   B. idx -> int16 wrapped layout via a tiny DRAM round-trip (2 DMAs),
      then ONE dma_gather fetches each query's 6KB candidate row.
   C. ACT squares (bias=-q) + DVE: d2 reduce, w=1/d2, max8, fused
      mask*w + sum, 3 fused flow dots, normalize.
"""

import os
import sys

import numpy as np

for _p in ("/opt/trn_rl_repo", "/root/.axon_site/_ro/trn_rl_repo"):
    if os.path.isdir(_p) and _p not in sys.path:
        sys.path.append(_p)

import ml_dtypes  # noqa: E402
from concourse import bass, mybir  # noqa: E402
from concourse import tile  # noqa: E402
from concourse.bass_utils import run_bass_kernel_spmd  # noqa: E402
from concourse import library_config  # noqa: E402

N_FULL = 32768
M_FULL = 16384
D = 3
K = 8
N_CORES = 8
P = 128

C = 2048          # anchors
L = 256           # candidates per anchor row
ROW = 6 * L       # fp32 elems per table row (planar x,y,z,fx,fy,fz)
S_EXP = 9         # score scale 2^s
QBITS = 11        # anchor idx bits
C1 = 1.5 * 2.0 ** (23 + QBITS)
OFF = 1.703125 * 2.0 ** 23
KM_ITERS = 4
KR = 19

_bf = lambda x: np.asarray(x, dtype=ml_dtypes.bfloat16)  # noqa: E731


def build_module(n_loc=N_FULL // N_CORES, stage=9):
    nt = n_loc // P
    assert n_loc % P == 0

    nc = bass.Bass("TRN2", debug=False)

    combA_d = nc.dram_tensor("combA", [32, C], mybir.dt.bfloat16,
                             kind="ExternalInput")
    combQ_d = nc.dram_tensor("combQ", [32, n_loc], mybir.dt.bfloat16,
                             kind="ExternalInput")
    nq_d = nc.dram_tensor("nq", [n_loc, D], mybir.dt.float32,
                          kind="ExternalInput")
    table_d = nc.dram_tensor("table", [C, ROW], mybir.dt.float32,
                             kind="ExternalInput")
    scr_d = nc.dram_tensor("scr", [4 * 16, 8], mybir.dt.int16,
                           kind="Internal")
    out_d = nc.dram_tensor("out", [n_loc, D], mybir.dt.float32,
                           kind="ExternalOutput")

    AF = mybir.ActivationFunctionType
    OP = mybir.AluOpType

    with tile.TileContext(nc) as tc:
        with (
            tc.tile_pool(name="const", bufs=1) as cpool,
            tc.tile_pool(name="psum", bufs=2, space="PSUM") as ppool,
            tc.tile_pool(name="gath", bufs=3) as gpool,
            tc.tile_pool(name="dsq", bufs=2) as dpool,
            tc.tile_pool(name="small", bufs=8) as mpool,
        ):
            nc.gpsimd.load_library(library_config.mlp)
            combA = cpool.tile([32, C], mybir.dt.bfloat16)
            nc.gpsimd.dma_start(out=combA[:, :], in_=combA_d[:, :])
            combQ = cpool.tile([32, n_loc], mybir.dt.bfloat16)
            nc.gpsimd.dma_start(out=combQ[:, :], in_=combQ_d[:, :])
            nq_all = cpool.tile([P, nt, D], mybir.dt.float32)
            nc.gpsimd.dma_start(
                out=nq_all[:, :, :],
                in_=nq_d[:, :].rearrange("(t p) d -> p t d", p=P),
            )
            out_all = cpool.tile([P, nt, D], mybir.dt.float32)
            idxT = [cpool.tile([P, 8], mybir.dt.int16, tag=f"idxT{i}",
                               name=f"idxT{i}")
                    for i in range(2)]
            for it in idxT:
                nc.vector.memset(it[...], 0)

            for t in range(nt):
                # --- stage A: packed anchor scores -> winner ---
                ps = ppool.tile([P, C], mybir.dt.float32, tag="ps")
                for cg in range(C // 512):
                    nc.tensor.matmul(
                        ps[:, cg * 512:(cg + 1) * 512],
                        lhsT=combQ[:KR, t * P:(t + 1) * P],
                        rhs=combA[:KR, cg * 512:(cg + 1) * 512],
                        start=True, stop=True,
                    )
                mval = mpool.tile([P, 1], mybir.dt.float32, tag="mval")
                nc.vector.tensor_reduce(
                    mval[:, :], ps[:, :], op=OP.max, axis=mybir.AxisListType.X
                )
                j32 = mpool.tile([P, 1], mybir.dt.uint32, tag="j32")
                nc.vector.tensor_scalar(
                    j32[:, :], mval[:, :].bitcast(mybir.dt.uint32),
                    (1 << QBITS) - 1, None, op0=OP.bitwise_and
                )
                x16 = mpool.tile([P, 1], mybir.dt.int16, tag="x16")
                nc.vector.tensor_copy(x16[:, :], j32[:, :])
                if stage < 2:
                    nc.vector.tensor_copy(out_all[:, t, 0:1], j32[:, :])
                    nc.vector.memset(out_all[:, t, 1:3], 0)
                    continue

                # --- idx wrap layout via DRAM round-trip ---
                scr = scr_d[(t % 4) * 16:(t % 4 + 1) * 16, :]
                nc.sync.dma_start(out=scr.rearrange("a b -> b a"),
                                  in_=x16[:, :])
                it = idxT[t % 2]
                nc.sync.dma_start(
                    out=it[:, :],
                    in_=scr.unsqueeze(0).broadcast_to([8, 16, 8]))

                # --- gather candidate rows ---
                gout = gpool.tile([P, 1, ROW], mybir.dt.float32, tag="gout")
                nc.gpsimd.dma_gather(
                    out_ap=gout[:, :, :],
                    in_ap=table_d[:, :],
                    idxs_ap=it[:, :],
                    num_idxs=P,
                    num_idxs_reg=P,
                    elem_size=ROW,
                )

                if stage < 3:
                    nc.vector.tensor_copy(out_all[:, t, :], gout[:, 0, 0:3])
                    continue
                # --- stage C: exact d2 + IDW ---
                dsq = dpool.tile([P, L, D], mybir.dt.float32, tag="dsq")
                for c in range(D):
                    nc.scalar.activation(
                        dsq[:, :, c], gout[:, 0, c * L:(c + 1) * L],
                        AF.Square, bias=nq_all[:, t, c:c + 1],
                    )
                d2 = mpool.tile([P, L], mybir.dt.float32, tag="d2")
                nc.vector.tensor_reduce(
                    d2[:, :], dsq[:, :, :], op=OP.add,
                    axis=mybir.AxisListType.X,
                )
                if stage < 4:
                    nc.vector.tensor_copy(out_all[:, t, :], d2[:, 0:3])
                    continue
                w = mpool.tile([P, L], mybir.dt.float32, tag="w")
                nc.vector.reciprocal(w[:, :], d2[:, :])
                v8 = mpool.tile([P, K], mybir.dt.float32, tag="v8")
                nc.vector.max(v8[:, :], w[:, :])
                if stage < 5:
                    nc.vector.tensor_copy(out_all[:, t, :], v8[:, 0:3])
                    continue
                maskw = mpool.tile([P, L], mybir.dt.float32, tag="maskw")
                acc = mpool.tile([P, 4], mybir.dt.float32, tag="acc")
                nc.vector.scalar_tensor_tensor(
                    maskw[:, :], w[:, :], v8[:, 7:8], w[:, :],
                    op0=OP.is_ge, op1=OP.mult, accum_out=acc[:, 3:4],
                )
                if stage < 6:
                    nc.vector.tensor_copy(out_all[:, t, :], maskw[:, 0:3])
                    continue
                junk = mpool.tile([P, L], mybir.dt.float32, tag="junk")
                for c in range(D):
                    nc.vector.scalar_tensor_tensor(
                        junk[:, :], maskw[:, :], 0.0,
                        gout[:, 0, (3 + c) * L:(4 + c) * L],
                        op0=OP.bypass, op1=OP.mult,
                        accum_out=acc[:, c:c + 1],
                    )
                wr = mpool.tile([P, 1], mybir.dt.float32, tag="wr")
                nc.vector.reciprocal(wr[:, :], acc[:, 3:4])
                nc.vector.tensor_scalar(
                    out_all[:, t, :], acc[:, 0:D], wr[:, 0:1], None,
                    op0=OP.mult,
                )

            nc.gpsimd.dma_start(
                out=out_d[:, :].rearrange("(t p) d -> p t d", p=P),
                in_=out_all[:, :, :],
            )

    mybir.codegen_inst_isa_subclasses(nc)
    _split_waits(nc)
    return nc


_SPLIT_SEQ = [0]


def _split_waits(nc, limit=1):
    """Move excess sem-waits onto preceding same-engine NOPs (several TRN2
    ISA structs accept only one sync-wait; see baseline kernel)."""
    import concourse.mybir as mybir  # noqa: PLC0415
    from concourse.tile_rust import add_dep_helper  # noqa: PLC0415

    for fn in nc.m.functions:
        for b in fn.blocks:
            il = b.instructions
            idx = 0
            while idx < len(il):
                inst = il[idx]
                si = inst.sync_info
                if si is not None and len(si.on_wait) > limit:
                    waits = list(si.on_wait)
                    excess, keep = waits[:-limit], waits[-limit:]
                    inst.sync_info = mybir.SyncInfo(
                        on_wait=keep, on_update=list(si.on_update)
                    )

                    def _safe_dep(a, b):
                        try:
                            add_dep_helper(a, b, True, "waitnop order")
                            return True
                        except ValueError:
                            return False

                    prev = None
                    for k in range(idx - 1, -1, -1):
                        if il[k].engine == inst.engine:
                            prev = il[k]
                            break
                    chain = prev
                    for j, wt in enumerate(excess):
                        _SPLIT_SEQ[0] += 1
                        nop = mybir.InstNoOp(
                            name=f"waitnop-{_SPLIT_SEQ[0]}", ins=[], outs=[]
                        )
                        nop.engine = inst.engine
                        nop.sync_info = mybir.SyncInfo(on_wait=[wt],
                                                       on_update=[])
                        nc.register_instruction(nop, overwrite=True)
                        if chain is not None:
                            _safe_dep(nop, chain)
                        chain = nop
                        il.insert(idx + j, nop)
                    _safe_dep(inst, chain)
                    idx += len(excess)
                idx += 1


# ---------------- host-side marshalling ----------------

def _morton_order(pts):
    lo, hi = pts.min(0), pts.max(0)
    g = np.clip(((pts - lo) / (hi - lo + 1e-9) * 1024).astype(np.uint64),
                0, 1023)

    def spread(x):
        x = (x | (x << 16)) & 0x030000FF
        x = (x | (x << 8)) & 0x0300F00F
        x = (x | (x << 4)) & 0x030C30C3
        x = (x | (x << 2)) & 0x09249249
        return x

    code = spread(g[:, 0]) | (spread(g[:, 1]) << 1) | (spread(g[:, 2]) << 2)
    return np.argsort(code, kind="stable")


def build_host_tables(r, f):
    r = np.asarray(r, np.float64)
    M = r.shape[0]
    cent = r[_morton_order(r)].reshape(C, M // C, 3).mean(1)
    for _ in range(KM_ITERS):
        d2 = ((r[:, None] - cent[None]) ** 2).sum(-1)
        a = np.argmin(d2, 1)
        sums = np.zeros((C, 3))
        cnts = np.zeros(C)
        np.add.at(sums, a, r)
        np.add.at(cnts, a, 1)
        nz = cnts > 0
        cent[nz] = sums[nz] / cnts[nz][:, None]

    d2cr = ((cent[:, None] - r[None]) ** 2).sum(-1)
    part = np.argpartition(d2cr, L, axis=1)
    lists = np.sort(part[:, :L], axis=1)
    RL2 = np.take_along_axis(d2cr, part[:, L - 1:L], 1)[:, 0]
    gam = 1.0 / RL2

    table = np.zeros((C, ROW), np.float32)
    f64 = np.asarray(f, np.float64)
    for c in range(3):
        table[:, c * L:(c + 1) * L] = r[lists, c]
        table[:, (3 + c) * L:(4 + c) * L] = f64[lists, c]
    return cent, gam, table


def _hl(x):
    h = _bf(x).astype(np.float64)
    return h, x - h


def build_rows(q, cent, gam):
    n = q.shape[0]
    q64 = np.asarray(q, np.float64)

    rows_ref = np.zeros((32, C))
    rows_q = np.zeros((32, n))

    t = -gam * (cent ** 2).sum(1) * 2.0 ** (S_EXP + QBITS)
    th, tl = _hl(t)
    rows_ref[0] = th; rows_q[0] = 1.0
    rows_ref[1] = tl; rows_q[1] = 1.0
    for c in range(3):
        v = 2.0 * gam * cent[:, c] * 2.0 ** (S_EXP + QBITS - 7)
        vh, vl = _hl(v)
        qh, ql = _hl(q64[:, c] * 2.0 ** 7)
        rows_ref[2 + 3 * c] = vh; rows_q[2 + 3 * c] = qh
        rows_ref[3 + 3 * c] = vl; rows_q[3 + 3 * c] = qh
        rows_ref[4 + 3 * c] = vh; rows_q[4 + 3 * c] = ql
    g2 = -gam * 2.0 ** (S_EXP + QBITS - 10)
    gh, gl = _hl(g2)
    qsq = (q64 ** 2).sum(1) * 2.0 ** 10
    qsh, qsl = _hl(qsq)
    rows_ref[11] = gh; rows_q[11] = qsh
    rows_ref[12] = gh; rows_q[12] = qsl
    rows_ref[13] = gl; rows_q[13] = qsh
    rows_ref[14] = C1; rows_q[14] = 1.0
    rows_ref[15] = -C1; rows_q[15] = 1.0
    rows_ref[16] = OFF; rows_q[16] = 1.0
    jg = np.arange(C, dtype=np.float64)
    jh, jl = _hl(jg)
    rows_ref[17] = jh; rows_q[17] = 1.0
    rows_ref[18] = jl; rows_q[18] = 1.0
    return _bf(rows_ref), _bf(rows_q)


def pack_inputs(query_points, ref_points, ref_flow):
    q = np.ascontiguousarray(np.asarray(query_points, dtype=np.float32))
    r0 = np.ascontiguousarray(np.asarray(ref_points, dtype=np.float32))
    f0 = np.ascontiguousarray(np.asarray(ref_flow, dtype=np.float32))
    n = q.shape[0]
    n_loc = n // N_CORES

    cent, gam, table = build_host_tables(r0, f0)
    rows_ref, rows_q = build_rows(q, cent, gam)

    nq = -q

    in_maps = []
    for core in range(N_CORES):
        sl = slice(core * n_loc, (core + 1) * n_loc)
        in_maps.append({
            "combA": rows_ref,
            "combQ": np.ascontiguousarray(rows_q[:, sl]),
            "nq": nq[sl],
            "table": table,
        })
    return in_maps


_NC_CACHE = {}


def _get_module(n_loc, stage=9):
    """Build + verify-compile (Tile scheduler is nondeterministic; retry
    until walrus accepts -- see baseline kernel)."""
    import tempfile

    from concourse.bass_utils import compile_bir_kernel

    key = (n_loc, stage)
    if key not in _NC_CACHE:
        last = None
        for _attempt in range(12):
            nc = build_module(n_loc, stage)
            try:
                with tempfile.TemporaryDirectory() as td:
                    compile_bir_kernel(nc.to_json_bytes(), td)
                _NC_CACHE[key] = nc
                break
            except Exception as e:  # noqa: BLE001 — retry on compile flake
                last = e
        else:
            raise RuntimeError(f"no schedule compiled after 12 tries: {last}")
    return _NC_CACHE[key]


def run_hw(query_points, ref_points, ref_flow, trace=False):
    in_maps = pack_inputs(query_points, ref_points, ref_flow)
    n = np.asarray(query_points).shape[0]
    nc = _get_module(n // N_CORES)
    res = run_bass_kernel_spmd(
        nc, in_maps, core_ids=list(range(N_CORES)), trace=trace
    )
    out = np.concatenate([r["out"] for r in res.results], axis=0)
    return out, res


def kernel(query_points, ref_points, ref_flow, power, k):
    assert int(power) == 2 and int(k) == K
    out, _ = run_hw(query_points, ref_points, ref_flow, trace=False)
    return out


# revision 5
# speedup vs baseline: 1.1202x; 1.1202x over previous
"""IDW k-NN flow interpolation via anchor-pruned candidate lists on 8
Trainium2 NeuronCores.

Sharding: queries split 8 ways (4096/core); anchor/table data replicated.

Algorithm (validated bit-exact in hostsim.py, rel ~1.1e-2):
  Host: k-means C=2048 anchors over refs; per-anchor list of the L=256
  refs nearest its centroid (gather table row, planar pts fp32 + flow
  fp32); per-anchor gamma = 1/RL^2 for scale-free normalized scoring.
  Device, per 128-query tile:
   A. PE bf16 packed matmul scores all anchors: P = OFF -
      round(2^s * |q-c|^2/RL_c^2)*2^11 + anchor_idx (C1-snap trick),
      DVE max-reduce -> winning anchor's packed score; idx = P mod 2048.
   B. idx -> int16 wrapped layout via a tiny DRAM round-trip (2 DMAs),
      then ONE dma_gather fetches each query's 6KB candidate row.
   C. ACT squares (bias=-q) + DVE: d2 reduce, w=1/d2, max8, fused
      mask*w + sum, 3 fused flow dots, normalize.
"""

import os
import sys

import numpy as np

for _p in ("/opt/trn_rl_repo", "/root/.axon_site/_ro/trn_rl_repo"):
    if os.path.isdir(_p) and _p not in sys.path:
        sys.path.append(_p)

import ml_dtypes  # noqa: E402
from concourse import bass, mybir  # noqa: E402
from concourse import tile  # noqa: E402
from concourse.bass_utils import run_bass_kernel_spmd  # noqa: E402
from concourse import library_config  # noqa: E402

N_FULL = 32768
M_FULL = 16384
D = 3
K = 8
N_CORES = 8
P = 128

C = 2048          # anchors
L = 256           # candidates per anchor row
ROW = 6 * L       # fp32 elems per table row (planar x,y,z,fx,fy,fz)
S_EXP = 9         # score scale 2^s
QBITS = 11        # anchor idx bits
C1 = 1.5 * 2.0 ** (23 + QBITS)
OFF = 1.703125 * 2.0 ** 23
KM_ITERS = 4
KR = 19

_bf = lambda x: np.asarray(x, dtype=ml_dtypes.bfloat16)  # noqa: E731


def build_module(n_loc=N_FULL // N_CORES, stage=9):
    nt = n_loc // P
    assert n_loc % P == 0

    nc = bass.Bass("TRN2", debug=False)

    combA_d = nc.dram_tensor("combA", [32, C], mybir.dt.bfloat16,
                             kind="ExternalInput")
    combQ_d = nc.dram_tensor("combQ", [32, n_loc], mybir.dt.bfloat16,
                             kind="ExternalInput")
    nq_d = nc.dram_tensor("nq", [n_loc, D], mybir.dt.float32,
                          kind="ExternalInput")
    table_d = nc.dram_tensor("table", [C, ROW], mybir.dt.float32,
                             kind="ExternalInput")
    scr_d = nc.dram_tensor("scr", [(n_loc // P // 4) * 16, 32],
                           mybir.dt.int16, kind="Internal")
    out_d = nc.dram_tensor("out", [n_loc, D], mybir.dt.float32,
                           kind="ExternalOutput")

    AF = mybir.ActivationFunctionType
    OP = mybir.AluOpType

    with tile.TileContext(nc) as tc:
        with (
            tc.tile_pool(name="const", bufs=1) as cpool,
            tc.tile_pool(name="psum", bufs=2, space="PSUM") as ppool,
            tc.tile_pool(name="gath", bufs=3) as gpool,
            tc.tile_pool(name="dsq", bufs=2) as dpool,
            tc.tile_pool(name="small", bufs=8) as mpool,
        ):
            nc.gpsimd.load_library(library_config.mlp)
            combA = cpool.tile([32, C], mybir.dt.bfloat16)
            nc.gpsimd.dma_start(out=combA[:, :], in_=combA_d[:, :])
            combQ = cpool.tile([32, n_loc], mybir.dt.bfloat16)
            nc.gpsimd.dma_start(out=combQ[:, :], in_=combQ_d[:, :])
            nq_all = cpool.tile([P, nt, D], mybir.dt.float32)
            nc.gpsimd.dma_start(
                out=nq_all[:, :, :],
                in_=nq_d[:, :].rearrange("(t p) d -> p t d", p=P),
            )
            out_all = cpool.tile([P, nt, D], mybir.dt.float32)
            idxT = [cpool.tile([P, 32], mybir.dt.int16, tag=f"idxT{i}",
                               name=f"idxT{i}")
                    for i in range(4)]
            for it in idxT:
                nc.vector.memset(it[...], 0)

            G = 4          # tiles per batch group
            assert nt % G == 0
            for g in range(nt // G):
                # --- stage A: packed anchor scores -> winners (G tiles) ---
                mval4 = mpool.tile([P, G], mybir.dt.float32, tag="mval4")
                for i in range(G):
                    t = g * G + i
                    ps = ppool.tile([P, C], mybir.dt.float32, tag="ps")
                    for cg in range(C // 512):
                        nc.tensor.matmul(
                            ps[:, cg * 512:(cg + 1) * 512],
                            lhsT=combQ[:KR, t * P:(t + 1) * P],
                            rhs=combA[:KR, cg * 512:(cg + 1) * 512],
                            start=True, stop=True,
                        )
                    nc.vector.tensor_reduce(
                        mval4[:, i:i + 1], ps[:, :], op=OP.max,
                        axis=mybir.AxisListType.X,
                    )
                j32 = mpool.tile([P, G], mybir.dt.uint32, tag="j32")
                nc.vector.tensor_scalar(
                    j32[:, :], mval4[:, :].bitcast(mybir.dt.uint32),
                    (1 << QBITS) - 1, None, op0=OP.bitwise_and
                )
                x16 = mpool.tile([P, G], mybir.dt.int16, tag="x16")
                nc.vector.tensor_copy(x16[:, :], j32[:, :])

                # --- idx wrap layout via DRAM round-trip (one per group) ---
                scr = scr_d[g * 16:(g + 1) * 16, :]
                nc.sync.dma_start(
                    out=scr.rearrange("a (c d) -> d a c", c=G, d=8),
                    in_=x16[:, :])
                it = idxT[g % 4]
                nc.sync.dma_start(
                    out=it[:, :],
                    in_=scr.unsqueeze(0).broadcast_to([8, 16, 8 * G]))

                # --- gather candidate rows for G tiles at once ---
                gout = gpool.tile([P, G, ROW], mybir.dt.float32, tag="gout")
                nc.gpsimd.dma_gather(
                    out_ap=gout[:, :, :],
                    in_ap=table_d[:, :],
                    idxs_ap=it[:, :],
                    num_idxs=P * G,
                    num_idxs_reg=P * G,
                    elem_size=ROW,
                )

                # --- stage C: exact d2 + IDW per tile ---
                sw4 = mpool.tile([P, G], mybir.dt.float32, tag="sw4")
                accs = []
                for i in range(G):
                    t = g * G + i
                    dsq = dpool.tile([P, L, D], mybir.dt.float32,
                                     tag=f"dsq{i % 2}", name=f"dsq{i % 2}")
                    for c in range(D):
                        nc.scalar.activation(
                            dsq[:, :, c], gout[:, i, c * L:(c + 1) * L],
                            AF.Square, bias=nq_all[:, t, c:c + 1],
                        )
                    d2 = mpool.tile([P, L], mybir.dt.float32, tag="d2")
                    nc.vector.tensor_reduce(
                        d2[:, :], dsq[:, :, :], op=OP.add,
                        axis=mybir.AxisListType.X,
                    )
                    w = mpool.tile([P, L], mybir.dt.float32, tag="w")
                    nc.vector.reciprocal(w[:, :], d2[:, :])
                    v8 = mpool.tile([P, K], mybir.dt.float32, tag="v8")
                    nc.vector.max(v8[:, :], w[:, :])
                    maskw = mpool.tile([P, L], mybir.dt.float32, tag="maskw")
                    acc = mpool.tile([P, D], mybir.dt.float32,
                                     tag=f"acc{i}", name=f"acc{i}")
                    accs.append(acc)
                    nc.vector.scalar_tensor_tensor(
                        maskw[:, :], w[:, :], v8[:, 7:8], w[:, :],
                        op0=OP.is_ge, op1=OP.mult, accum_out=sw4[:, i:i + 1],
                    )
                    junk = mpool.tile([P, L], mybir.dt.float32, tag="junk")
                    for c in range(D):
                        nc.vector.scalar_tensor_tensor(
                            junk[:, :], maskw[:, :], 0.0,
                            gout[:, i, (3 + c) * L:(4 + c) * L],
                            op0=OP.bypass, op1=OP.mult,
                            accum_out=acc[:, c:c + 1],
                        )
                wr4 = mpool.tile([P, G], mybir.dt.float32, tag="wr4")
                nc.vector.reciprocal(wr4[:, :], sw4[:, :])
                for i in range(G):
                    t = g * G + i
                    nc.vector.tensor_scalar(
                        out_all[:, t, :], accs[i][:, 0:D], wr4[:, i:i + 1],
                        None, op0=OP.mult,
                    )

            nc.gpsimd.dma_start(
                out=out_d[:, :].rearrange("(t p) d -> p t d", p=P),
                in_=out_all[:, :, :],
            )

    mybir.codegen_inst_isa_subclasses(nc)
    _split_waits(nc)
    return nc


_SPLIT_SEQ = [0]


def _split_waits(nc, limit=1):
    """Move excess sem-waits onto preceding same-engine NOPs (several TRN2
    ISA structs accept only one sync-wait; see baseline kernel)."""
    import concourse.mybir as mybir  # noqa: PLC0415
    from concourse.tile_rust import add_dep_helper  # noqa: PLC0415

    for fn in nc.m.functions:
        for b in fn.blocks:
            il = b.instructions
            idx = 0
            while idx < len(il):
                inst = il[idx]
                si = inst.sync_info
                if si is not None and len(si.on_wait) > limit:
                    waits = list(si.on_wait)
                    excess, keep = waits[:-limit], waits[-limit:]
                    inst.sync_info = mybir.SyncInfo(
                        on_wait=keep, on_update=list(si.on_update)
                    )

                    def _safe_dep(a, b):
                        try:
                            add_dep_helper(a, b, True, "waitnop order")
                            return True
                        except ValueError:
                            return False

                    prev = None
                    for k in range(idx - 1, -1, -1):
                        if il[k].engine == inst.engine:
                            prev = il[k]
                            break
                    chain = prev
                    for j, wt in enumerate(excess):
                        _SPLIT_SEQ[0] += 1
                        nop = mybir.InstNoOp(
                            name=f"waitnop-{_SPLIT_SEQ[0]}", ins=[], outs=[]
                        )
                        nop.engine = inst.engine
                        nop.sync_info = mybir.SyncInfo(on_wait=[wt],
                                                       on_update=[])
                        nc.register_instruction(nop, overwrite=True)
                        if chain is not None:
                            _safe_dep(nop, chain)
                        chain = nop
                        il.insert(idx + j, nop)
                    _safe_dep(inst, chain)
                    idx += len(excess)
                idx += 1


# ---------------- host-side marshalling ----------------

def _morton_order(pts):
    lo, hi = pts.min(0), pts.max(0)
    g = np.clip(((pts - lo) / (hi - lo + 1e-9) * 1024).astype(np.uint64),
                0, 1023)

    def spread(x):
        x = (x | (x << 16)) & 0x030000FF
        x = (x | (x << 8)) & 0x0300F00F
        x = (x | (x << 4)) & 0x030C30C3
        x = (x | (x << 2)) & 0x09249249
        return x

    code = spread(g[:, 0]) | (spread(g[:, 1]) << 1) | (spread(g[:, 2]) << 2)
    return np.argsort(code, kind="stable")


def build_host_tables(r, f):
    r = np.asarray(r, np.float64)
    M = r.shape[0]
    cent = r[_morton_order(r)].reshape(C, M // C, 3).mean(1)
    for _ in range(KM_ITERS):
        d2 = ((r[:, None] - cent[None]) ** 2).sum(-1)
        a = np.argmin(d2, 1)
        sums = np.zeros((C, 3))
        cnts = np.zeros(C)
        np.add.at(sums, a, r)
        np.add.at(cnts, a, 1)
        nz = cnts > 0
        cent[nz] = sums[nz] / cnts[nz][:, None]

    d2cr = ((cent[:, None] - r[None]) ** 2).sum(-1)
    part = np.argpartition(d2cr, L, axis=1)
    lists = np.sort(part[:, :L], axis=1)
    RL2 = np.take_along_axis(d2cr, part[:, L - 1:L], 1)[:, 0]
    gam = 1.0 / RL2

    table = np.zeros((C, ROW), np.float32)
    f64 = np.asarray(f, np.float64)
    for c in range(3):
        table[:, c * L:(c + 1) * L] = r[lists, c]
        table[:, (3 + c) * L:(4 + c) * L] = f64[lists, c]
    return cent, gam, table


def _hl(x):
    h = _bf(x).astype(np.float64)
    return h, x - h


def build_rows(q, cent, gam):
    n = q.shape[0]
    q64 = np.asarray(q, np.float64)

    rows_ref = np.zeros((32, C))
    rows_q = np.zeros((32, n))

    t = -gam * (cent ** 2).sum(1) * 2.0 ** (S_EXP + QBITS)
    th, tl = _hl(t)
    rows_ref[0] = th; rows_q[0] = 1.0
    rows_ref[1] = tl; rows_q[1] = 1.0
    for c in range(3):
        v = 2.0 * gam * cent[:, c] * 2.0 ** (S_EXP + QBITS - 7)
        vh, vl = _hl(v)
        qh, ql = _hl(q64[:, c] * 2.0 ** 7)
        rows_ref[2 + 3 * c] = vh; rows_q[2 + 3 * c] = qh
        rows_ref[3 + 3 * c] = vl; rows_q[3 + 3 * c] = qh
        rows_ref[4 + 3 * c] = vh; rows_q[4 + 3 * c] = ql
    g2 = -gam * 2.0 ** (S_EXP + QBITS - 10)
    gh, gl = _hl(g2)
    qsq = (q64 ** 2).sum(1) * 2.0 ** 10
    qsh, qsl = _hl(qsq)
    rows_ref[11] = gh; rows_q[11] = qsh
    rows_ref[12] = gh; rows_q[12] = qsl
    rows_ref[13] = gl; rows_q[13] = qsh
    rows_ref[14] = C1; rows_q[14] = 1.0
    rows_ref[15] = -C1; rows_q[15] = 1.0
    rows_ref[16] = OFF; rows_q[16] = 1.0
    jg = np.arange(C, dtype=np.float64)
    jh, jl = _hl(jg)
    rows_ref[17] = jh; rows_q[17] = 1.0
    rows_ref[18] = jl; rows_q[18] = 1.0
    return _bf(rows_ref), _bf(rows_q)


def pack_inputs(query_points, ref_points, ref_flow):
    q = np.ascontiguousarray(np.asarray(query_points, dtype=np.float32))
    r0 = np.ascontiguousarray(np.asarray(ref_points, dtype=np.float32))
    f0 = np.ascontiguousarray(np.asarray(ref_flow, dtype=np.float32))
    n = q.shape[0]
    n_loc = n // N_CORES

    cent, gam, table = build_host_tables(r0, f0)
    rows_ref, rows_q = build_rows(q, cent, gam)

    nq = -q

    in_maps = []
    for core in range(N_CORES):
        sl = slice(core * n_loc, (core + 1) * n_loc)
        in_maps.append({
            "combA": rows_ref,
            "combQ": np.ascontiguousarray(rows_q[:, sl]),
            "nq": nq[sl],
            "table": table,
        })
    return in_maps


_NC_CACHE = {}


def _get_module(n_loc, stage=9):
    """Build + verify-compile (Tile scheduler is nondeterministic; retry
    until walrus accepts -- see baseline kernel)."""
    import tempfile

    from concourse.bass_utils import compile_bir_kernel

    key = (n_loc, stage)
    if key not in _NC_CACHE:
        last = None
        for _attempt in range(12):
            nc = build_module(n_loc, stage)
            try:
                with tempfile.TemporaryDirectory() as td:
                    compile_bir_kernel(nc.to_json_bytes(), td)
                _NC_CACHE[key] = nc
                break
            except Exception as e:  # noqa: BLE001 — retry on compile flake
                last = e
        else:
            raise RuntimeError(f"no schedule compiled after 12 tries: {last}")
    return _NC_CACHE[key]


def run_hw(query_points, ref_points, ref_flow, trace=False):
    in_maps = pack_inputs(query_points, ref_points, ref_flow)
    n = np.asarray(query_points).shape[0]
    nc = _get_module(n // N_CORES)
    res = run_bass_kernel_spmd(
        nc, in_maps, core_ids=list(range(N_CORES)), trace=trace
    )
    out = np.concatenate([r["out"] for r in res.results], axis=0)
    return out, res


def kernel(query_points, ref_points, ref_flow, power, k):
    assert int(power) == 2 and int(k) == K
    out, _ = run_hw(query_points, ref_points, ref_flow, trace=False)
    return out
